# revision 20
# baseline (speedup 1.0000x reference)
"""Trainium2 Bass kernel for MiniSelfAttention.

Shapes (full problem): x (4, 2048, 1024), Wq/Wk/Wv/Wo (1024, 1024), bo (1024,).
H=16 heads, D=64. out = softmax(q k^T / 8) v  projected by Wo.

Sharding across 8 cores: core c -> batch b = c//2, head-group g = c%2
(8 heads = 512 features per group).  Each core computes a partial output
projection (its 512 ctx features x Wo slice); host sums the two partials
per batch and adds the bias.

Schedule design (per core): the scalar engine's exp over 8 heads x T^2
scores (33.5M elems, ~1.06us per [128,1024] op, 256 ops = ~272us) is the
hard floor; the kernel keeps ACT busy back-to-back by flattening the
attention into one (pair p, q-block j, key-tile s) stream and emitting
all projection work (qkv/out matmuls) as credit-based "fillers" inside
the attention loop's PE slack.  attn@v is software-pipelined LAG tiles
behind the scores/exp stream so a late v tile never blocks scores.

Per-core math (T=2048, V=1024, F=512, D=64, H=8):
  kT/qT[p] = (Wk/Wq chunk p @ x.T)    [128, T]  (pair p = 2 heads)
  vS[t]    = x_tile @ Wv.T            [128, 512]
  scores sc[s,tq] per pair: 2 row-tiled (tile_position) matmuls, D=64 each
  e = exp(sc/8)  one ACT op per (p,j,s), no max subtraction (scores~N(0,1))
  ctx: one [128,512] psum per (p,j); the pair's 2 heads are col-tiled
    (tile_position (0,0)/(0,64), M=64) and accumulate v^T e concurrently
  Z via DVE: E_acc[128,1024] (fp16) += e per s-tile; then
    bcZ[64,512] = ones[128,64]^T @ E_acc_half  (partition sum, broadcast)
    ctxT = pcx / bcZ  (single DVE divide per head)
  out rows = ctxT.T @ WoT (emitted as fillers during last pair + tail)
"""

import sys

sys.path.insert(0, "/opt/trn_rl_repo")

from collections import deque

import numpy as np

import concourse.bacc as bacc
import concourse.mybir as mybir
from concourse import tile
from concourse.bass_utils import run_bass_kernel_spmd

F32 = mybir.dt.float32
BF16 = mybir.dt.bfloat16
FP16 = mybir.dt.float16
AF = mybir.ActivationFunctionType
ALU = mybir.AluOpType

DIM = 1024
HEADS = 16
D = 64
N_CORES = 8


class Fillers:
    """Ordered queue of emission generators, drained by PE-time credit.

    A generator may carry a `gate`: it will not be pulled before the given
    global iteration.  This keeps double-buffer prefills from being emitted
    while the previous occupant still has unemitted readers (which would
    silently rebind those readers' data dependencies to the new contents).
    """

    def __init__(self):
        self.q = deque()
        self.gens = {}
        self.gates = {}

    def add(self, key, gen, gate=0):
        self.gens[key] = gen
        self.gates[key] = gate
        self.q.append(key)

    def pull(self, credit, gi=1 << 30):
        while credit > 0 and self.q:
            key = self.q[0]
            if self.gates.get(key, 0) > gi:
                return
            try:
                credit -= next(self.gens[key])
            except StopIteration:
                del self.gens[key]
                self.q.popleft()

    def force(self, key):
        gen = self.gens.pop(key, None)
        if gen is None:
            return
        for _ in gen:
            pass
        self.q.remove(key)


def build_nc(T=2048, V=DIM, F=512, mmdt=BF16):
    H = F // D                # heads per core (8)
    NP = H // 2               # head pairs (4)
    KC = V // 128             # contraction chunks (8)
    NT = T // 128             # 128-row tiles of T (16)
    TQ = 512                  # q block
    NJ = T // TQ              # q blocks (4)
    NS = T // 128             # key tiles (16)
    KF = F // 128             # ctx feature chunks (4)
    LAG = 3                   # attn@v pipeline lag (score iters)
    EB = 8                    # e-tile buffers
    CREDIT = 2.0              # filler matmul-units per attn iteration

    nc = bacc.Bacc(trn_type="TRN2")
    xT = nc.dram_tensor("xT", [V, T], mmdt, kind="ExternalInput")
    wqT = nc.dram_tensor("wqT", [V, F], mmdt, kind="ExternalInput")
    wkT = nc.dram_tensor("wkT", [V, F], mmdt, kind="ExternalInput")
    wvT = nc.dram_tensor("wvT", [V, F], mmdt, kind="ExternalInput")
    woT = nc.dram_tensor("woT", [F, V], mmdt, kind="ExternalInput")
    out = nc.dram_tensor("out", [T, V], F32, kind="ExternalOutput")

    with tile.TileContext(nc) as tc:
        with (
            tc.tile_pool(name="sb", bufs=1) as pp,
            tc.tile_pool(name="eb", bufs=1) as pe_,
            tc.tile_pool(name="ps", bufs=1, space="PSUM") as pps,
        ):
            # ---- persistent SBUF ----
            # x as two half-T tiles per k-chunk so consumers of the first half
            # start as soon as it lands.  Submissions split between the SP and
            # ACT queues (ACT's input submits all land before its first exp).
            HT = T // 2
            xtA = [pp.tile([128, HT], mmdt, tag=f"xA{k}", name=f"xA{k}") for k in range(KC)]
            xtB = [pp.tile([128, HT], mmdt, tag=f"xB{k}", name=f"xB{k}") for k in range(KC)]
            wks = [pp.tile([128, F], mmdt, tag=f"wk{k}", name=f"wk{k}") for k in range(KC)]
            wqs = [pp.tile([128, F], mmdt, tag=f"wq{k}", name=f"wq{k}") for k in range(KC)]
            wvs = [pp.tile([128, F], mmdt, tag=f"wv{k}", name=f"wv{k}") for k in range(KC)]
            wos = [pp.tile([128, V], mmdt, tag=f"wo{k}", name=f"wo{k}") for k in range(KF)]
            # Never submit DMAs from the ACT queue (a pseudo-DMA occupies the
            # issuing engine for the whole transfer).  Need-order across the
            # SP and GpSimd queues: kT chain needs xtA+wk, qT(0,0) needs wq.
            for k in range(KC):
                nc.sync.dma_start(xtA[k][:], xT[128 * k : 128 * (k + 1), 0:HT])
            for k in range(KC):
                nc.gpsimd.dma_start(wqs[k][:], wqT[128 * k : 128 * (k + 1), :])
            for k in range(KC):
                nc.sync.dma_start(wks[k][:], wkT[128 * k : 128 * (k + 1), :])
            for k in range(KC):
                nc.gpsimd.dma_start(xtB[k][:], xT[128 * k : 128 * (k + 1), HT:T])
            for k in range(KC):
                nc.gpsimd.dma_start(wvs[k][:], wvT[128 * k : 128 * (k + 1), :])
            for k in range(KF):
                nc.sync.dma_start(wos[k][:], woT[128 * k : 128 * (k + 1), :])

            def xap(k, c0, c1):
                if c1 <= HT:
                    return xtA[k][:, c0:c1]
                return xtB[k][:, c0 - HT : c1 - HT]

            qTb = [pp.tile([128, T], mmdt, tag=f"qTb{i}", name=f"qTb{i}") for i in range(2)]
            kTb = [pp.tile([128, T], mmdt, tag=f"kTb{i}", name=f"kTb{i}") for i in range(2)]
            vS = [pp.tile([128, F], mmdt, tag=f"vS{t}", name=f"vS{t}") for t in range(NT)]
            ctxT = [pp.tile([128, T], mmdt, tag=f"cT{p}", name=f"cT{p}") for p in range(NP)]
            onesC = pp.tile([128, D], FP16, tag="onesC", name="onesC")
            nc.vector.memset(onesC[:], 1.0)

            # ---- emission generators (filler units ~ one N=512 matmul) ----
            def g_qT(p, j):
                ps = pps.tile([128, TQ], F32, tag="mm", bufs=2, name="psq")
                for k in range(KC):
                    nc.tensor.matmul(
                        ps[:],
                        wqs[k][:, 128 * p : 128 * (p + 1)],
                        xap(k, TQ * j, TQ * (j + 1)),
                        start=(k == 0),
                        stop=(k == KC - 1),
                    )
                    yield 1
                nc.vector.tensor_copy(qTb[p % 2][:, TQ * j : TQ * (j + 1)], ps[:])
                yield 0.5

            def g_kT(p, n):
                ps = pps.tile([128, TQ], F32, tag="mm", bufs=2, name="psk")
                for k in range(KC):
                    nc.tensor.matmul(
                        ps[:],
                        wks[k][:, 128 * p : 128 * (p + 1)],
                        xap(k, TQ * n, TQ * (n + 1)),
                        start=(k == 0),
                        stop=(k == KC - 1),
                    )
                    yield 1
                nc.vector.tensor_copy(kTb[p % 2][:, TQ * n : TQ * (n + 1)], ps[:])
                yield 0.5

            def g_v(t):
                ps = pps.tile([128, F], F32, tag="mm", bufs=2, name="psv")
                for k in range(KC):
                    nc.tensor.matmul(
                        ps[:],
                        xap(k, 128 * t, 128 * (t + 1)),
                        wvs[k][:],
                        start=(k == 0),
                        stop=(k == KC - 1),
                    )
                    yield 1
                nc.vector.tensor_copy(vS[t][:], ps[:])
                yield 0.5

            def g_out(t):
                ot = pp.tile([128, V], F32, tag="ot", bufs=3, name="ot")
                for n in range(2):
                    ps = pps.tile([128, 512], F32, tag="mm", bufs=2, name="pso")
                    for kf in range(KF):
                        nc.tensor.matmul(
                            ps[:],
                            ctxT[kf][:, 128 * t : 128 * (t + 1)],
                            wos[kf][:, 512 * n : 512 * (n + 1)],
                            start=(kf == 0),
                            stop=(kf == KF - 1),
                        )
                        yield 1
                    nc.vector.tensor_copy(ot[:, 512 * n : 512 * (n + 1)], ps[:])
                    yield 0.5
                # split the 512KB row-block store across DMA engines (tail tiles
                # 4-way: the very last store otherwise dominates the epilogue)
                nsp = 4 if t >= NT - 4 else 2
                w = V // nsp
                for i in range(nsp):
                    nc.sync.dma_start(
                        out[128 * t : 128 * (t + 1), w * i : w * (i + 1)],
                        ot[:, w * i : w * (i + 1)],
                    )
                yield 0.3

            fill = Fillers()

            def drain(gen):
                for _ in gen:
                    pass

            # ---- prologue: kT p0 (full), qT p0 j0 ----
            for n in range(NJ):
                drain(g_kT(0, n))
            drain(g_qT(0, 0))

            # ---- filler queue (global need-order) ----
            for t in range(8):
                fill.add(("v", t), g_v(t))
            fill.add(("qT", 0, 1), g_qT(0, 1))
            for t in range(8, 12):
                fill.add(("v", t), g_v(t))
            fill.add(("qT", 0, 2), g_qT(0, 2))
            for t in range(12, 16):
                fill.add(("v", t), g_v(t))
            fill.add(("qT", 0, 3), g_qT(0, 3))
            for p in range(1, NP):
                # pair p reuses the (p-2) buffers: do not emit its prefill
                # until group p-1 starts (all group p-2 readers emitted).
                gate = max(0, (p - 1) * NJ * NS)
                for n in range(NJ):
                    fill.add(("kT", p, n), g_kT(p, n), gate=gate)
                for j in range(NJ):
                    fill.add(("qT", p, j), g_qT(p, j), gate=gate)

            # ---- attention stream ----
            pcx_live = {}
            eacc_live = {}

            def emit_attnv(p, j, s, e):
                if p == 0 and j == 0:
                    fill.force(("v", s))
                if s == 0:
                    pcx_live[(p, j)] = pps.tile(
                        [128, TQ], F32, tag="cx", bufs=2, name="pcx"
                    )
                    eacc_live[(p, j)] = pe_.tile(
                        [128, 2 * TQ], FP16, tag="eacc", bufs=2, name="eacc"
                    )
                pcx = pcx_live[(p, j)]
                ea = eacc_live[(p, j)]
                for half in range(2):
                    h = 2 * p + half
                    nc.tensor.matmul(
                        pcx[64 * half : 64 * (half + 1), :],
                        vS[s][:, 64 * h : 64 * h + 64],
                        e[:, TQ * half : TQ * (half + 1)],
                        tile_position=(0, 64 * half),
                        start=(s == 0),
                        stop=(s == NS - 1),
                    )
                # Z-accumulation split across DVE and the otherwise-idle GpSimd
                # (independent column chains, no cross-engine dependency).
                ESP = 768
                if s == 0:
                    nc.vector.tensor_copy(ea[:, 0:ESP], e[:, 0:ESP])
                    nc.gpsimd.tensor_copy(ea[:, ESP:], e[:, ESP:])
                else:
                    nc.vector.tensor_add(ea[:, 0:ESP], ea[:, 0:ESP], e[:, 0:ESP])
                    nc.gpsimd.tensor_add(ea[:, ESP:], ea[:, ESP:], e[:, ESP:])
                if s == NS - 1:
                    emit_norm(p, j)

            def emit_norm(p, j):
                pcx = pcx_live.pop((p, j))
                ea = eacc_live.pop((p, j))
                for half in range(2):
                    # bcZ[m, tq] = sum_i E_acc[i, tq]  (partition-sum broadcast
                    # to 64 rows in one matmul)
                    bz = pps.tile([64, TQ], F32, tag="mm", bufs=2, name="bz")
                    nc.tensor.matmul(
                        bz[:],
                        onesC[:],
                        ea[:, TQ * half : TQ * (half + 1)],
                    )
                    bzs = pp.tile([64, TQ], F32, tag="bzs", bufs=2, name="bzs")
                    nc.vector.tensor_copy(bzs[:], bz[:])
                    rb = pp.tile([64, TQ], F32, tag="rb", bufs=2, name="rb")
                    nc.vector.reciprocal_approx_fast(rb[:], bzs[:])
                    nc.vector.tensor_mul(
                        ctxT[p][64 * half : 64 * (half + 1), TQ * j : TQ * (j + 1)],
                        pcx[64 * half : 64 * (half + 1), :],
                        rb[:],
                    )
                if p == NP - 1:
                    for ti in range(4):
                        fill.add(("out", 4 * j + ti), g_out(4 * j + ti))

            pend = deque()
            for gi in range(NP * NJ * NS):
                p, j, s = gi // (NJ * NS), (gi // NS) % NJ, gi % NS
                if s == 0:
                    if j == 0 and p > 0:
                        fill.force(("kT", p, 0))
                    fill.force(("qT", p, j))
                elif s % 4 == 0 and j == 0 and p > 0:
                    fill.force(("kT", p, s // 4))  # JIT: key block for s..s+3
                # attn@v for iteration gi-LAG first: its exp finished an
                # iteration ago, so it never stalls the PE FIFO ahead of the
                # scores that feed the next exp.
                if len(pend) >= LAG:
                    emit_attnv(*pend.popleft())
                # scores: two row-tiled 64-contraction matmuls, concurrent on PE
                sc = pps.tile([128, 2 * TQ], F32, tag="sc", bufs=2, name="sc")
                for half in range(2):
                    lo = 64 * half
                    nc.tensor.matmul(
                        sc[:, TQ * half : TQ * (half + 1)],
                        kTb[p % 2][lo : lo + 64, 128 * s : 128 * (s + 1)],
                        qTb[p % 2][lo : lo + 64, TQ * j : TQ * (j + 1)],
                        tile_position=(lo, 0),
                    )
                e = pe_.tile([128, 2 * TQ], mmdt, tag="e", bufs=EB, name="e")
                nc.scalar.activation(e[:], sc[:], AF.Exp, scale=1.0 / np.sqrt(D))
                pend.append((p, j, s, e))
                fill.pull(CREDIT, gi)
            while pend:
                emit_attnv(*pend.popleft())
            # tail: drain all remaining fillers (incl. out j3)
            while fill.q:
                fill.pull(1000.0)

    nc.compile()
    return nc


_NC_CACHE = {}


def _get_nc(T=2048, V=DIM, F=512):
    key = (T, V, F)
    if key not in _NC_CACHE:
        _NC_CACHE[key] = build_nc(T, V, F)
    return _NC_CACHE[key]


def make_in_maps(x, Wq, Wk, Wv, Wo, np_mmdt):
    B = x.shape[0]
    F = Wq.shape[0] // 2
    in_maps = []
    for c in range(N_CORES):
        b, g = divmod(c, 2)
        rows = slice(g * F, (g + 1) * F)
        in_maps.append(
            {
                "xT": np.ascontiguousarray(x[b].T).astype(np_mmdt),
                "wqT": np.ascontiguousarray(Wq[rows].T).astype(np_mmdt),
                "wkT": np.ascontiguousarray(Wk[rows].T).astype(np_mmdt),
                "wvT": np.ascontiguousarray(Wv[rows].T).astype(np_mmdt),
                "woT": np.ascontiguousarray(Wo[:, rows].T).astype(np_mmdt),
            }
        )
    return in_maps


def kernel(x, Wq, Wk, Wv, Wo, bo, trace=False):
    x = np.asarray(x, np.float32)
    B, T, V = x.shape
    nc = _get_nc(T=T, V=V, F=V // 2)
    np_mmdt = mybir.dt.np(BF16)
    in_maps = make_in_maps(
        x,
        np.asarray(Wq, np.float32),
        np.asarray(Wk, np.float32),
        np.asarray(Wv, np.float32),
        np.asarray(Wo, np.float32),
        np_mmdt,
    )
    res = run_bass_kernel_spmd(nc, in_maps, core_ids=list(range(N_CORES)), trace=trace)
    outs = [r["out"] for r in res.results]
    full = np.empty((B, T, V), np.float32)
    for b in range(B):
        full[b] = outs[2 * b] + outs[2 * b + 1] + np.asarray(bo, np.float32)
    if trace:
        kernel.last_exec_time_ns = res.exec_time_ns
        kernel.last_results = res
    return full


# revision 23
# speedup vs baseline: 1.0861x; 1.0861x over previous
"""Trainium2 Bass kernel for MiniSelfAttention.

Shapes (full problem): x (4, 2048, 1024), Wq/Wk/Wv/Wo (1024, 1024), bo (1024,).
H=16 heads, D=64. out = softmax(q k^T / 8) v  projected by Wo.

Sharding across 8 cores: core c -> batch b = c//2, head-group g = c%2
(8 heads = 512 features per group).  Each core computes a partial output
projection (its 512 ctx features x Wo slice); host sums the two partials
per batch and adds the bias.

Schedule design (per core): the scalar engine's exp over 8 heads x T^2
scores (33.5M elems, ~1.06us per [128,1024] op, 256 ops = ~272us) is the
hard floor; the kernel keeps ACT busy back-to-back by flattening the
attention into one (pair p, q-block j, key-tile s) stream and emitting
all projection work (qkv/out matmuls) as credit-based "fillers" inside
the attention loop's PE slack.  attn@v is software-pipelined LAG tiles
behind the scores/exp stream so a late v tile never blocks scores.

Per-core math (T=2048, V=1024, F=512, D=64, H=8):
  kT/qT[p] = (Wk/Wq chunk p @ x.T)    [128, T]  (pair p = 2 heads)
  vS[t]    = x_tile @ Wv.T            [128, 512]
  scores sc[s,tq] per pair: 2 row-tiled (tile_position) matmuls, D=64 each
  e = exp(sc/8)  one ACT op per (p,j,s), no max subtraction (scores~N(0,1))
  ctx: one [128,512] psum per (p,j); the pair's 2 heads are col-tiled
    (tile_position (0,0)/(0,64), M=64) and accumulate v^T e concurrently
  Z via DVE: E_acc[128,1024] (fp16) += e per s-tile; then
    bcZ[64,512] = ones[128,64]^T @ E_acc_half  (partition sum, broadcast)
    ctxT = pcx / bcZ  (single DVE divide per head)
  out rows = ctxT.T @ WoT (emitted as fillers during last pair + tail)
"""

import sys

sys.path.insert(0, "/opt/trn_rl_repo")

from collections import deque

import numpy as np

import concourse.bacc as bacc
import concourse.mybir as mybir
from concourse import tile
from concourse.bass_utils import run_bass_kernel_spmd

F32 = mybir.dt.float32
BF16 = mybir.dt.bfloat16
FP16 = mybir.dt.float16
AF = mybir.ActivationFunctionType
ALU = mybir.AluOpType

DIM = 1024
HEADS = 16
D = 64
N_CORES = 8


class Fillers:
    """Ordered queue of emission generators, drained by PE-time credit.

    A generator may carry a `gate`: it will not be pulled before the given
    global iteration.  This keeps double-buffer prefills from being emitted
    while the previous occupant still has unemitted readers (which would
    silently rebind those readers' data dependencies to the new contents).
    """

    def __init__(self):
        self.q = deque()
        self.gens = {}
        self.gates = {}

    def add(self, key, gen, gate=0):
        self.gens[key] = gen
        self.gates[key] = gate
        self.q.append(key)

    def pull(self, credit, gi=1 << 30):
        while credit > 0 and self.q:
            key = self.q[0]
            if self.gates.get(key, 0) > gi:
                return
            try:
                credit -= next(self.gens[key])
            except StopIteration:
                del self.gens[key]
                self.q.popleft()

    def force(self, key):
        gen = self.gens.pop(key, None)
        if gen is None:
            return
        for _ in gen:
            pass
        self.q.remove(key)


def build_nc(T=2048, V=DIM, F=512, mmdt=BF16):
    H = F // D                # heads per core (8)
    NP = H // 2               # head pairs (4)
    KC = V // 128             # contraction chunks (8)
    NT = T // 128             # 128-row tiles of T (16)
    TQ = 512                  # q block
    NJ = T // TQ              # q blocks (4)
    NS = T // 128             # key tiles (16)
    KF = F // 128             # ctx feature chunks (4)
    LAG = 3                   # attn@v pipeline lag (score iters)
    EB = 8                    # e-tile buffers
    CREDIT = 2.0              # filler matmul-units per attn iteration

    nc = bacc.Bacc(trn_type="TRN2")
    xT = nc.dram_tensor("xT", [V, T], mmdt, kind="ExternalInput")
    wqT = nc.dram_tensor("wqT", [V, F], mmdt, kind="ExternalInput")
    wkT = nc.dram_tensor("wkT", [V, F], mmdt, kind="ExternalInput")
    wvT = nc.dram_tensor("wvT", [V, F], mmdt, kind="ExternalInput")
    woT = nc.dram_tensor("woT", [F, V], mmdt, kind="ExternalInput")
    out = nc.dram_tensor("out", [T, V], F32, kind="ExternalOutput")

    with tile.TileContext(nc) as tc:
        with (
            tc.tile_pool(name="sb", bufs=1) as pp,
            tc.tile_pool(name="eb", bufs=1) as pe_,
            tc.tile_pool(name="ps", bufs=1, space="PSUM") as pps,
        ):
            # ---- persistent SBUF ----
            # x as two half-T tiles per k-chunk so consumers of the first half
            # start as soon as it lands.  Submissions split between the SP and
            # ACT queues (ACT's input submits all land before its first exp).
            # x as four 512-col tiles per k-chunk ([128,512]=128KB, 5.7us per
            # DMA) so early consumers start as soon as their quarter lands.
            # Never submit DMAs from the ACT queue (a pseudo-DMA occupies the
            # issuing engine for the whole transfer).  Need-order across the
            # SP and GpSimd queues: kT chain needs xq0+wk, qT(0,0) needs wq.
            xq = [
                [
                    pp.tile([128, TQ], mmdt, tag=f"xq{q}_{k}", name=f"xq{q}_{k}")
                    for k in range(KC)
                ]
                for q in range(4)
            ]
            wks = [pp.tile([128, F], mmdt, tag=f"wk{k}", name=f"wk{k}") for k in range(KC)]
            wqs = [pp.tile([128, F], mmdt, tag=f"wq{k}", name=f"wq{k}") for k in range(KC)]
            wvs = [pp.tile([128, F], mmdt, tag=f"wv{k}", name=f"wv{k}") for k in range(KC)]
            wos = [pp.tile([128, V], mmdt, tag=f"wo{k}", name=f"wo{k}") for k in range(KF)]

            def _xdma(eng, q, k):
                eng.dma_start(
                    xq[q][k][:], xT[128 * k : 128 * (k + 1), TQ * q : TQ * (q + 1)]
                )

            for k in range(KC):
                _xdma(nc.sync, 0, k)
            for k in range(KC):
                nc.gpsimd.dma_start(wqs[k][:], wqT[128 * k : 128 * (k + 1), :])
            for k in range(KC):
                nc.sync.dma_start(wks[k][:], wkT[128 * k : 128 * (k + 1), :])
            for k in range(KC):
                nc.gpsimd.dma_start(wvs[k][:], wvT[128 * k : 128 * (k + 1), :])
            for k in range(KC):
                _xdma(nc.sync, 1, k)
            for k in range(KC):
                _xdma(nc.gpsimd, 2, k)
            for k in range(KC):
                _xdma(nc.gpsimd, 3, k)
            for k in range(KF):
                nc.sync.dma_start(wos[k][:], woT[128 * k : 128 * (k + 1), :])

            def xap(k, c0, c1):
                q = c0 // TQ
                assert c1 <= TQ * (q + 1)
                return xq[q][k][:, c0 - TQ * q : c1 - TQ * q]

            qTb = [pp.tile([128, T], mmdt, tag=f"qTb{i}", name=f"qTb{i}") for i in range(2)]
            kTb = [pp.tile([128, T], mmdt, tag=f"kTb{i}", name=f"kTb{i}") for i in range(2)]
            vS = [pp.tile([128, F], mmdt, tag=f"vS{t}", name=f"vS{t}") for t in range(NT)]
            ctxT = [pp.tile([128, T], mmdt, tag=f"cT{p}", name=f"cT{p}") for p in range(NP)]
            onesC = pp.tile([128, D], FP16, tag="onesC", name="onesC")
            nc.vector.memset(onesC[:], 1.0)

            # ---- emission generators (filler units ~ one N=512 matmul) ----
            def g_qT(p, j):
                ps = pps.tile([128, TQ], F32, tag="mm", bufs=2, name="psq")
                for k in range(KC):
                    nc.tensor.matmul(
                        ps[:],
                        wqs[k][:, 128 * p : 128 * (p + 1)],
                        xap(k, TQ * j, TQ * (j + 1)),
                        start=(k == 0),
                        stop=(k == KC - 1),
                    )
                    yield 1
                nc.vector.tensor_copy(qTb[p % 2][:, TQ * j : TQ * (j + 1)], ps[:])
                yield 0.5

            def g_kT(p, n):
                ps = pps.tile([128, TQ], F32, tag="mm", bufs=2, name="psk")
                for k in range(KC):
                    nc.tensor.matmul(
                        ps[:],
                        wks[k][:, 128 * p : 128 * (p + 1)],
                        xap(k, TQ * n, TQ * (n + 1)),
                        start=(k == 0),
                        stop=(k == KC - 1),
                    )
                    yield 1
                nc.vector.tensor_copy(kTb[p % 2][:, TQ * n : TQ * (n + 1)], ps[:])
                yield 0.5

            def g_v(t):
                ps = pps.tile([128, F], F32, tag="mm", bufs=2, name="psv")
                for k in range(KC):
                    nc.tensor.matmul(
                        ps[:],
                        xap(k, 128 * t, 128 * (t + 1)),
                        wvs[k][:],
                        start=(k == 0),
                        stop=(k == KC - 1),
                    )
                    yield 1
                nc.vector.tensor_copy(vS[t][:], ps[:])
                yield 0.5

            def g_out(t):
                ot = pp.tile([128, V], F32, tag="ot", bufs=3, name="ot")
                for n in range(2):
                    ps = pps.tile([128, 512], F32, tag="mm", bufs=2, name="pso")
                    for kf in range(KF):
                        nc.tensor.matmul(
                            ps[:],
                            ctxT[kf][:, 128 * t : 128 * (t + 1)],
                            wos[kf][:, 512 * n : 512 * (n + 1)],
                            start=(kf == 0),
                            stop=(kf == KF - 1),
                        )
                        yield 1
                    nc.vector.tensor_copy(ot[:, 512 * n : 512 * (n + 1)], ps[:])
                    yield 0.5
                # split the 512KB row-block store across DMA engines (tail tiles
                # 4-way: the very last store otherwise dominates the epilogue)
                nsp = 4 if t >= NT - 4 else 2
                w = V // nsp
                for i in range(nsp):
                    nc.sync.dma_start(
                        out[128 * t : 128 * (t + 1), w * i : w * (i + 1)],
                        ot[:, w * i : w * (i + 1)],
                    )
                yield 0.3

            fill = Fillers()

            def drain(gen):
                for _ in gen:
                    pass

            # ---- prologue: kT(0,0) and qT(0,0) first (they gate the first
            # exp); the later key blocks trickle in as their x quarter lands.
            drain(g_kT(0, 0))
            drain(g_qT(0, 0))
            for n in range(1, NJ):
                drain(g_kT(0, n))

            # ---- filler queue (global need-order) ----
            for t in range(8):
                fill.add(("v", t), g_v(t))
            fill.add(("qT", 0, 1), g_qT(0, 1))
            for t in range(8, 12):
                fill.add(("v", t), g_v(t))
            fill.add(("qT", 0, 2), g_qT(0, 2))
            for t in range(12, 16):
                fill.add(("v", t), g_v(t))
            fill.add(("qT", 0, 3), g_qT(0, 3))
            for p in range(1, NP):
                # pair p reuses the (p-2) buffers: do not emit its prefill
                # until group p-1 starts (all group p-2 readers emitted).
                gate = max(0, (p - 1) * NJ * NS)
                for n in range(NJ):
                    fill.add(("kT", p, n), g_kT(p, n), gate=gate)
                for j in range(NJ):
                    fill.add(("qT", p, j), g_qT(p, j), gate=gate)

            # ---- attention stream ----
            pcx_live = {}
            eacc_live = {}

            def emit_attnv(p, j, s, e):
                if p == 0 and j == 0:
                    fill.force(("v", s))
                if s == 0:
                    pcx_live[(p, j)] = pps.tile(
                        [128, TQ], F32, tag="cx", bufs=2, name="pcx"
                    )
                    eacc_live[(p, j)] = pe_.tile(
                        [128, 2 * TQ], FP16, tag="eacc", bufs=2, name="eacc"
                    )
                pcx = pcx_live[(p, j)]
                ea = eacc_live[(p, j)]
                for half in range(2):
                    h = 2 * p + half
                    nc.tensor.matmul(
                        pcx[64 * half : 64 * (half + 1), :],
                        vS[s][:, 64 * h : 64 * h + 64],
                        e[:, TQ * half : TQ * (half + 1)],
                        tile_position=(0, 64 * half),
                        start=(s == 0),
                        stop=(s == NS - 1),
                    )
                if s == 0:
                    nc.vector.tensor_copy(ea[:], e[:])
                else:
                    nc.vector.tensor_add(ea[:], ea[:], e[:])
                if s == NS - 1:
                    emit_norm(p, j)

            def emit_norm(p, j):
                pcx = pcx_live.pop((p, j))
                ea = eacc_live.pop((p, j))
                for half in range(2):
                    # bcZ[m, tq] = sum_i E_acc[i, tq]  (partition-sum broadcast
                    # to 64 rows in one matmul)
                    bz = pps.tile([64, TQ], F32, tag="mm", bufs=2, name="bz")
                    nc.tensor.matmul(
                        bz[:],
                        onesC[:],
                        ea[:, TQ * half : TQ * (half + 1)],
                    )
                    bzs = pp.tile([64, TQ], F32, tag="bzs", bufs=2, name="bzs")
                    nc.vector.tensor_copy(bzs[:], bz[:])
                    rb = pp.tile([64, TQ], F32, tag="rb", bufs=2, name="rb")
                    nc.vector.reciprocal_approx_fast(rb[:], bzs[:])
                    nc.vector.tensor_mul(
                        ctxT[p][64 * half : 64 * (half + 1), TQ * j : TQ * (j + 1)],
                        pcx[64 * half : 64 * (half + 1), :],
                        rb[:],
                    )
                if p == NP - 1:
                    for ti in range(4):
                        fill.add(("out", 4 * j + ti), g_out(4 * j + ti))

            pend = deque()
            for gi in range(NP * NJ * NS):
                p, j, s = gi // (NJ * NS), (gi // NS) % NJ, gi % NS
                if s == 0:
                    if j == 0 and p > 0:
                        fill.force(("kT", p, 0))
                    fill.force(("qT", p, j))
                elif s % 4 == 0 and j == 0 and p > 0:
                    fill.force(("kT", p, s // 4))  # JIT: key block for s..s+3
                # attn@v for iteration gi-LAG first: its exp finished an
                # iteration ago, so it never stalls the PE FIFO ahead of the
                # scores that feed the next exp.
                if len(pend) >= LAG:
                    emit_attnv(*pend.popleft())
                # scores: two row-tiled 64-contraction matmuls, concurrent on PE
                sc = pps.tile([128, 2 * TQ], F32, tag="sc", bufs=2, name="sc")
                for half in range(2):
                    lo = 64 * half
                    nc.tensor.matmul(
                        sc[:, TQ * half : TQ * (half + 1)],
                        kTb[p % 2][lo : lo + 64, 128 * s : 128 * (s + 1)],
                        qTb[p % 2][lo : lo + 64, TQ * j : TQ * (j + 1)],
                        tile_position=(lo, 0),
                    )
                e = pe_.tile([128, 2 * TQ], mmdt, tag="e", bufs=EB, name="e")
                nc.scalar.activation(e[:], sc[:], AF.Exp, scale=1.0 / np.sqrt(D))
                pend.append((p, j, s, e))
                fill.pull(CREDIT, gi)
            while pend:
                emit_attnv(*pend.popleft())
            # tail: drain all remaining fillers (incl. out j3)
            while fill.q:
                fill.pull(1000.0)

    nc.compile()
    return nc


_NC_CACHE = {}


def _get_nc(T=2048, V=DIM, F=512):
    key = (T, V, F)
    if key not in _NC_CACHE:
        _NC_CACHE[key] = build_nc(T, V, F)
    return _NC_CACHE[key]


def make_in_maps(x, Wq, Wk, Wv, Wo, np_mmdt):
    B = x.shape[0]
    F = Wq.shape[0] // 2
    in_maps = []
    for c in range(N_CORES):
        b, g = divmod(c, 2)
        rows = slice(g * F, (g + 1) * F)
        in_maps.append(
            {
                "xT": np.ascontiguousarray(x[b].T).astype(np_mmdt),
                "wqT": np.ascontiguousarray(Wq[rows].T).astype(np_mmdt),
                "wkT": np.ascontiguousarray(Wk[rows].T).astype(np_mmdt),
                "wvT": np.ascontiguousarray(Wv[rows].T).astype(np_mmdt),
                "woT": np.ascontiguousarray(Wo[:, rows].T).astype(np_mmdt),
            }
        )
    return in_maps


def kernel(x, Wq, Wk, Wv, Wo, bo, trace=False):
    x = np.asarray(x, np.float32)
    B, T, V = x.shape
    nc = _get_nc(T=T, V=V, F=V // 2)
    np_mmdt = mybir.dt.np(BF16)
    in_maps = make_in_maps(
        x,
        np.asarray(Wq, np.float32),
        np.asarray(Wk, np.float32),
        np.asarray(Wv, np.float32),
        np.asarray(Wo, np.float32),
        np_mmdt,
    )
    res = run_bass_kernel_spmd(nc, in_maps, core_ids=list(range(N_CORES)), trace=trace)
    outs = [r["out"] for r in res.results]
    full = np.empty((B, T, V), np.float32)
    for b in range(B):
        full[b] = outs[2 * b] + outs[2 * b + 1] + np.asarray(bo, np.float32)
    if trace:
        kernel.last_exec_time_ns = res.exec_time_ns
        kernel.last_results = res
    return full
